# revision 1
# baseline (speedup 1.0000x reference)
"""Trainium2 Bass kernel for nn_Enhanced_transformer (dense transformer block).

Strategy
--------
Data-parallel: batch B=8 -> one batch element per NeuronCore (8 cores), no
collectives. Per core, everything runs in channel-major ("transposed") layout
[channel-part, token-free], which makes every GEMM contraction land on the
partition dim with zero runtime transposes:

  h^T = LN1(x)^T           stats via ones-matmul partition reduction
  x_v^T = v_wT @ h^T       (f32r)        -> spill to DRAM
  x_q   = h^T' @ qk_wT     (fp32, [n-part, q-free])
  energy= x_q' @ x_q       (fp32, PSUM-accumulated across chunks)
  A1    = energy @ t1_wT   (fp32)  + t1_b (free-bcast)  -> gelu
  att2  = t2_wT' @ A1      (fp32)  + t2_b (part bias)   -> softmax -> f32r
  t_out^T = att' @ x_v^T   (f32r);  x1^T = t_out^T + x^T
  h2^T  = LN2(x1)^T        -> spill;  x1 -> spill
  m     = gelu(m1_wT' @ h2^T + m1_b);  out = m2_wT' @ m + m2_b + x1^T

dtypes: attention-logits chain exact fp32 (4 cyc/row but tiny FLOPs);
all big GEMMs float32r (tf32-like, 1 cyc/row at free>=256 == bf16 speed).
Expected absmax error vs fp32 reference ~7e-3 (simulated).

Host side: per-core transposes of x / weights; output transposed back.
"""

import numpy as np

import concourse.bass as bass
import concourse.tile as tile
from concourse import bacc, mybir
from concourse import bass_utils

F32 = mybir.dt.float32
F32R = mybir.dt.float32r
AF = mybir.ActivationFunctionType
ALU = mybir.AluOpType
AX = mybir.AxisListType

B, N, P = 8, 4096, 1024
P4 = P // 4          # 256
EPS = 1e-5
CH = 512             # token chunk
NCH = N // CH        # 8
KP = P // 128        # 8 channel tiles
KQ = P4 // 128       # 2


def _build(apply_ln1_affine: bool, apply_ln2_affine: bool, loop_R: int = 1):
    nc = bacc.Bacc("TRN2", target_bir_lowering=False, debug=False)

    # ---- DRAM I/O ----
    xT_d = nc.dram_tensor("xT", [P, N], F32, kind="ExternalInput").ap()
    qk_wT_d = nc.dram_tensor("qk_wT", [P, P4], F32, kind="ExternalInput").ap()
    v_wT_d = nc.dram_tensor("v_wT", [P, P], F32R, kind="ExternalInput").ap()
    t1_wT_d = nc.dram_tensor("t1_wT", [P4, P], F32, kind="ExternalInput").ap()
    t2_wT_d = nc.dram_tensor("t2_wT", [P4, P], F32, kind="ExternalInput").ap()
    m1_wT_d = nc.dram_tensor("m1_wT", [P, P], F32R, kind="ExternalInput").ap()
    m2_wT_d = nc.dram_tensor("m2_wT", [P, P], F32R, kind="ExternalInput").ap()
    v_b_d = nc.dram_tensor("v_b", [P], F32, kind="ExternalInput").ap()
    t1_b_d = nc.dram_tensor("t1_b", [P], F32, kind="ExternalInput").ap()
    t2_b_d = nc.dram_tensor("t2_b", [P], F32, kind="ExternalInput").ap()
    m1_b_d = nc.dram_tensor("m1_b", [P], F32, kind="ExternalInput").ap()
    m2_b_d = nc.dram_tensor("m2_b", [P], F32, kind="ExternalInput").ap()
    ln_d = {}
    if apply_ln1_affine:
        ln_d["ln1_g"] = nc.dram_tensor("ln1_g", [P], F32, kind="ExternalInput").ap()
        ln_d["ln1_b"] = nc.dram_tensor("ln1_b", [P], F32, kind="ExternalInput").ap()
    if apply_ln2_affine:
        ln_d["ln2_g"] = nc.dram_tensor("ln2_g", [P], F32, kind="ExternalInput").ap()
        ln_d["ln2_b"] = nc.dram_tensor("ln2_b", [P], F32, kind="ExternalInput").ap()
    outT_d = nc.dram_tensor("outT", [P, N], F32, kind="ExternalOutput").ap()

    def part_bias_tiles(pool, dram_ap, name):
        """[P] dram vector -> list of KP [128,1] per-partition SBUF tiles."""
        tiles = []
        for t in range(KP):
            bt = pool.tile([128, 1], F32, tag=f"{name}{t}", name=f"{name}{t}")
            nc.scalar.dma_start(bt[:], dram_ap[t * 128 : (t + 1) * 128])
            tiles.append(bt)
        return tiles

    with tile.TileContext(nc) as tc:
        with (
            tc.tile_pool(name="dram", bufs=1, space="DRAM") as dram_pool,
            tc.tile_pool(name="consts", bufs=1) as consts,
        ):
            xv_sp = dram_pool.tile([P, N], F32R, name="xv_sp")
            h2_sp = dram_pool.tile([P, N], F32R, name="h2_sp")
            x1_sp = dram_pool.tile([P, N], F32, name="x1_sp")
            att_sp = dram_pool.tile([P, P], F32R, name="att_sp")

            ones_f = consts.tile([128, 128], F32, tag="ones_f", name="ones_f")
            nc.vector.memset(ones_f[:], 1.0 / P)
            ones_r = consts.tile([128, 128], F32R, tag="ones_r", name="ones_r")
            nc.vector.tensor_copy(ones_r[:], ones_f[:])
            eps_t = consts.tile([128, 1], F32, tag="eps", name="eps_t")
            nc.vector.memset(eps_t[:], EPS)

            vb_t = part_bias_tiles(consts, v_b_d, "vb")
            t2b_t = part_bias_tiles(consts, t2_b_d, "t2b")
            m1b_t = part_bias_tiles(consts, m1_b_d, "m1b")
            m2b_t = part_bias_tiles(consts, m2_b_d, "m2b")
            # t1_b broadcast along partitions: [128, P] via stride-0 DMA
            t1b_bc = consts.tile([128, P], F32, tag="t1b_bc", name="t1b_bc")
            t1b_src = bass.AP(
                tensor=t1_b_d.tensor, offset=t1_b_d.offset,
                ap=[[0, 128], *t1_b_d.ap],
            )
            nc.scalar.dma_start(t1b_bc[:], t1b_src)
            ln_t = {}
            if apply_ln1_affine:
                ln_t["g1"] = part_bias_tiles(consts, ln_d["ln1_g"], "g1")
                ln_t["b1"] = part_bias_tiles(consts, ln_d["ln1_b"], "b1")
            if apply_ln2_affine:
                ln_t["g2"] = part_bias_tiles(consts, ln_d["ln2_g"], "g2")
                ln_t["b2"] = part_bias_tiles(consts, ln_d["ln2_b"], "b2")

            def ln_stats(psP, pP, x_tiles, sq_tiles, tag):
                """x_tiles: KP x [128,CH] fp32; sq_tiles f32r. -> (mu_b, rho_b)
                [128,CH] fp32, already broadcast to all partitions (full ones
                matrix in the matmul replicates the row-sums)."""
                ps_s = psP.tile([128, CH], F32, tag=f"{tag}_s", name=f"{tag}_s")
                ps_q = psP.tile([128, CH], F32, tag=f"{tag}_q", name=f"{tag}_q")
                for p in range(KP):
                    nc.tensor.matmul(
                        ps_s[:], ones_r[:], x_tiles[p][:],
                        start=(p == 0), stop=(p == KP - 1),
                    )
                for p in range(KP):
                    nc.tensor.matmul(
                        ps_q[:], ones_r[:], sq_tiles[p][:],
                        start=(p == 0), stop=(p == KP - 1),
                    )
                mu_b = pP.tile([128, CH], F32, tag=f"{tag}_mu", name=f"{tag}_mu")
                nc.vector.tensor_copy(mu_b[:], ps_s[:])
                var = pP.tile([128, CH], F32, tag=f"{tag}_var", name=f"{tag}_var")
                nc.vector.tensor_mul(var[:], mu_b[:], mu_b[:])
                nc.vector.tensor_tensor(var[:], ps_q[:], var[:], ALU.subtract)
                nc.scalar.activation(var[:], var[:], AF.Sqrt, bias=eps_t[:])
                rho_b = pP.tile([128, CH], F32, tag=f"{tag}_rho", name=f"{tag}_rho")
                nc.vector.reciprocal(rho_b[:], var[:])
                return mu_b, rho_b

            # Optional hardware repeat-loop for timing (test.py only).
            from contextlib import ExitStack as _ES
            _loop_ctx = _ES()
            if loop_R > 1:
                _loop_ctx.enter_context(tc.For_i(0, loop_R, 1))
            # ============ PHASE A: LN1, x_v, x_q, energy ============
            with tc.tile_pool(name="psE", bufs=1, space="PSUM") as psE:
                e_ps = [psE.tile([128, P4], F32, tag=f"e{i}", name=f"e_ps{i}")
                        for i in range(KQ)]
                with (
                    tc.tile_pool(name="wA", bufs=1) as wA,
                    tc.tile_pool(name="pA", bufs=1) as pA,
                    tc.tile_pool(name="psA", bufs=1, space="PSUM") as psA,
                ):
                    v_w_r = []
                    for p in range(KP):
                        wr = wA.tile([128, P], F32R, tag=f"vw{p}", name=f"vw{p}")
                        nc.scalar.dma_start(wr[:], v_wT_d[p * 128 : (p + 1) * 128, :])
                        v_w_r.append(wr)
                    qk_w_t = []
                    for p in range(KP):
                        wt = wA.tile([128, P4], F32, tag=f"qkw{p}", name=f"qkw{p}")
                        nc.scalar.dma_start(wt[:], qk_wT_d[p * 128 : (p + 1) * 128, :])
                        qk_w_t.append(wt)

                    for c in range(NCH):
                        cs = slice(c * CH, (c + 1) * CH)
                        xt = []
                        for p in range(KP):
                            t = pA.tile([128, CH], F32, tag=f"xt{p}", name=f"xt{p}",
                                        bufs=2)
                            nc.sync.dma_start(t[:], xT_d[p * 128 : (p + 1) * 128, cs])
                            xt.append(t)
                        xr, sq = [], []
                        for p in range(KP):
                            r = pA.tile([128, CH], F32R, tag=f"xr{p}", name=f"xr{p}")
                            nc.gpsimd.tensor_copy(r[:], xt[p][:])
                            xr.append(r)
                            s = pA.tile([128, CH], F32R, tag=f"sq{p}", name=f"sq{p}")
                            nc.scalar.activation(s[:], xt[p][:], AF.Square)
                            sq.append(s)
                        mu_b, rho_b = ln_stats(psA, pA, xr, sq, "st1")

                        h32, h_r = [], []
                        for p in range(KP):
                            h = pA.tile([128, CH], F32, tag=f"h32{p}", name=f"h32{p}",
                                        bufs=2)
                            nc.vector.tensor_tensor(h[:], xt[p][:], mu_b[:],
                                                    ALU.subtract)
                            nc.vector.tensor_mul(h[:], h[:], rho_b[:])
                            if apply_ln1_affine:
                                nc.scalar.activation(
                                    h[:], h[:], AF.Identity,
                                    bias=ln_t["b1"][p][:], scale=ln_t["g1"][p][:],
                                )
                            hr = pA.tile([128, CH], F32R, tag=f"hr{p}", name=f"hr{p}",
                                         bufs=2)
                            nc.scalar.activation(hr[:], h[:], AF.Copy)
                            h32.append(h)
                            h_r.append(hr)

                        # x_q chunk + energy accumulation
                        for ns in range(CH // 128):
                            ps = psA.tile([128, P4], F32, tag="xq", name="xq_ps",
                                          bufs=2)
                            for p in range(KP):
                                nc.tensor.matmul(
                                    ps[:],
                                    h32[p][:, ns * 128 : (ns + 1) * 128],
                                    qk_w_t[p][:],
                                    start=(p == 0), stop=(p == KP - 1),
                                )
                            xq = pA.tile([128, P4], F32, tag="xqs", name="xqs",
                                         bufs=3)
                            nc.vector.tensor_copy(xq[:], ps[:])
                            first = c == 0 and ns == 0
                            last = c == NCH - 1 and ns == CH // 128 - 1
                            for qh in range(KQ):
                                nc.tensor.matmul(
                                    e_ps[qh][:],
                                    xq[:, qh * 128 : (qh + 1) * 128],
                                    xq[:],
                                    start=first, stop=last,
                                    skip_group_check=True,
                                )

                        # x_v^T chunk
                        for o in range(KP):
                            ps = psA.tile([128, CH], F32, tag="xv", name="xv_ps",
                                          bufs=2)
                            for p in range(KP):
                                nc.tensor.matmul(
                                    ps[:], v_w_r[p][:, o * 128 : (o + 1) * 128],
                                    h_r[p][:], start=(p == 0), stop=(p == KP - 1),
                                )
                            xv = pA.tile([128, CH], F32R, tag="xvs", name="xvs",
                                         bufs=2)
                            nc.scalar.activation(
                                xv[:], ps[:], AF.Identity, bias=vb_t[o][:]
                            )
                            nc.sync.dma_start(
                                xv_sp[o * 128 : (o + 1) * 128, cs], xv[:]
                            )

                # ============ PHASE B: logits + softmax ============
                with (
                    tc.tile_pool(name="wB", bufs=1) as wB,
                    tc.tile_pool(name="pB", bufs=1) as pB,
                    tc.tile_pool(name="psB", bufs=1, space="PSUM") as psB,
                ):
                    t1_w_t, t2_w_t = [], []
                    for qh in range(KQ):
                        wt = wB.tile([128, P], F32, tag=f"t1w{qh}", name=f"t1w{qh}")
                        nc.scalar.dma_start(wt[:],
                                            t1_wT_d[qh * 128 : (qh + 1) * 128, :])
                        t1_w_t.append(wt)
                        wt2 = wB.tile([128, P], F32, tag=f"t2w{qh}", name=f"t2w{qh}")
                        nc.scalar.dma_start(wt2[:],
                                            t2_wT_d[qh * 128 : (qh + 1) * 128, :])
                        t2_w_t.append(wt2)
                    energy_sb = []
                    for qh in range(KQ):
                        e = wB.tile([128, P4], F32, tag=f"esb{qh}", name=f"esb{qh}")
                        nc.vector.tensor_copy(e[:], e_ps[qh][:])
                        energy_sb.append(e)

                    # A1[b, a] = sum_q energy[q,b] t1_wT[q,a]; +t1_b[a]; gelu
                    a1g = []
                    for bh in range(KQ):
                        a1 = pB.tile([128, P], F32, tag=f"a1_{bh}", name=f"a1_{bh}")
                        for oc in range(P // 512):
                            ps = psB.tile([128, 512], F32, tag="a1", name="a1_ps",
                                          bufs=2)
                            for qh in range(KQ):
                                nc.tensor.matmul(
                                    ps[:],
                                    energy_sb[qh][:, bh * 128 : (bh + 1) * 128],
                                    t1_w_t[qh][:, oc * 512 : (oc + 1) * 512],
                                    start=(qh == 0), stop=(qh == KQ - 1),
                                )
                            nc.vector.tensor_tensor(
                                a1[:, oc * 512 : (oc + 1) * 512], ps[:],
                                t1b_bc[:, oc * 512 : (oc + 1) * 512], ALU.add,
                            )
                        ag = wB.tile([128, P], F32, tag=f"a1g{bh}", name=f"a1g{bh}")
                        nc.scalar.activation(ag[:], a1[:], AF.Gelu)
                        a1g.append(ag)

                    # att2 + softmax -> att_r (f32r)
                    for o in range(KP):
                        att2 = pB.tile([128, P], F32, tag="att2", name="att2",
                                       bufs=2)
                        for kc in range(P // 512):
                            ps = psB.tile([128, 512], F32, tag="a2", name="a2_ps",
                                          bufs=2)
                            for ph in range(KQ):
                                nc.tensor.matmul(
                                    ps[:],
                                    t2_w_t[ph][:, o * 128 : (o + 1) * 128],
                                    a1g[ph][:, kc * 512 : (kc + 1) * 512],
                                    start=(ph == 0), stop=(ph == KQ - 1),
                                )
                            nc.scalar.activation(
                                att2[:, kc * 512 : (kc + 1) * 512], ps[:],
                                AF.Identity, bias=t2b_t[o][:],
                            )
                        negmax = pB.tile([128, 1], F32, tag="negmax", name="negmax",
                                         bufs=2)
                        nc.vector.tensor_reduce(
                            negmax[:], att2[:], axis=AX.X, op=ALU.max, negate=True
                        )
                        esum = pB.tile([128, 1], F32, tag="esum", name="esum",
                                       bufs=2)
                        expv = pB.tile([128, P], F32, tag="expv", name="expv",
                                       bufs=2)
                        nc.scalar.activation(
                            expv[:], att2[:], AF.Exp, bias=negmax[:],
                            accum_out=esum[:],
                        )
                        rec = pB.tile([128, 1], F32, tag="rec", name="rec", bufs=2)
                        nc.vector.reciprocal(rec[:], esum[:])
                        ar = pB.tile([128, P], F32R, tag="att_t", name="att_t",
                                     bufs=2)
                        nc.vector.tensor_scalar_mul(ar[:], expv[:], rec[:])
                        nc.sync.dma_start(
                            att_sp[o * 128 : (o + 1) * 128, :], ar[:]
                        )

            # ============ PHASE C1: t_out, x1, LN2, h2 ============
            with (
                tc.tile_pool(name="pC1", bufs=1) as pC,
                tc.tile_pool(name="psC1", bufs=1, space="PSUM") as psC,
            ):
                att_r = []
                for p in range(KP):
                    ar = pC.tile([128, P], F32R, tag=f"att{p}", name=f"att{p}")
                    nc.scalar.dma_start(ar[:], att_sp[p * 128 : (p + 1) * 128, :])
                    att_r.append(ar)

                def c1_tout_q(c, q, xt, xv):
                    cs = slice(c * CH, (c + 1) * CH)
                    nb = 1
                    ps = psC.tile([128, CH], F32, tag="tout", name="tout_ps",
                                  bufs=4)
                    for p in range(KP):
                        nc.tensor.matmul(
                            ps[:],
                            att_r[p][:, q * 128 : (q + 1) * 128],
                            xv[p][:],
                            start=(p == 0), stop=(p == KP - 1),
                        )
                    x1 = pC.tile([128, CH], F32, tag=f"x1{q}", name=f"x1{q}",
                                 bufs=nb)
                    nc.vector.tensor_tensor(x1[:], ps[:], xt[q][:], ALU.add)
                    nc.sync.dma_start(x1_sp[q * 128 : (q + 1) * 128, cs], x1[:])
                    r = pC.tile([128, CH], F32R, tag=f"x1r{q}", name=f"x1r{q}",
                                bufs=nb)
                    nc.gpsimd.tensor_copy(r[:], x1[:])
                    sq = pC.tile([128, CH], F32R, tag=f"sq2{q}", name=f"sq2{q}",
                                 bufs=nb)
                    nc.scalar.activation(sq[:], x1[:], AF.Square)
                    return x1, r, sq

                def c1_stats_h2(c, x1f, x1r, sq2):
                    cs = slice(c * CH, (c + 1) * CH)
                    mu2, rho2 = ln_stats(psC, pC, x1r, sq2, "st2")
                    for p in range(KP):
                        nc.vector.tensor_tensor(x1f[p][:], x1f[p][:], mu2[:],
                                                ALU.subtract)
                        h2r = pC.tile([128, CH], F32R, tag=f"h2r{p}",
                                      name=f"h2r{p}", bufs=2)
                        nc.vector.tensor_mul(h2r[:], x1f[p][:], rho2[:])
                        if apply_ln2_affine:
                            nc.scalar.activation(
                                h2r[:], h2r[:], AF.Identity,
                                bias=ln_t["b2"][p][:], scale=ln_t["g2"][p][:],
                            )
                        nc.sync.dma_start(
                            h2_sp[p * 128 : (p + 1) * 128, cs], h2r[:]
                        )

                for c in range(NCH):
                    cs = slice(c * CH, (c + 1) * CH)
                    xt = []
                    for p in range(KP):
                        t = pC.tile([128, CH], F32, tag=f"xt{p}", name=f"xt{p}",
                                    bufs=2)
                        nc.sync.dma_start(t[:], xT_d[p * 128 : (p + 1) * 128, cs])
                        xt.append(t)
                    xv = []
                    for p in range(KP):
                        t = pC.tile([128, CH], F32R, tag=f"xv{p}", name=f"xv{p}",
                                    bufs=2)
                        nc.sync.dma_start(t[:],
                                          xv_sp[p * 128 : (p + 1) * 128, cs])
                        xv.append(t)

                    x1f, x1r, sq2 = [], [], []
                    for q in range(KP):
                        a, b_, d = c1_tout_q(c, q, xt, xv)
                        x1f.append(a); x1r.append(b_); sq2.append(d)
                    c1_stats_h2(c, x1f, x1r, sq2)

            # ============ PHASE C2: MLP + final residual ============
            with (
                tc.tile_pool(name="wC2", bufs=1) as wC,
                tc.tile_pool(name="pC2", bufs=1) as pC2,
                tc.tile_pool(name="psC2", bufs=1, space="PSUM") as psC2,
            ):
                m1_w_r, m2_w_r = [], []
                for p in range(KP):
                    wr = wC.tile([128, P], F32R, tag=f"m1w{p}", name=f"m1w{p}")
                    nc.scalar.dma_start(wr[:], m1_wT_d[p * 128 : (p + 1) * 128, :])
                    m1_w_r.append(wr)
                for p in range(KP):
                    wr = wC.tile([128, P], F32R, tag=f"m2w{p}", name=f"m2w{p}")
                    nc.sync.dma_start(wr[:], m2_wT_d[p * 128 : (p + 1) * 128, :])
                    m2_w_r.append(wr)

                def c2_m1(c):
                    cs = slice(c * CH, (c + 1) * CH)
                    h2 = []
                    for p in range(KP):
                        t = pC2.tile([128, CH], F32R, tag=f"h2{p}", name=f"h2{p}",
                                     bufs=2)
                        nc.sync.dma_start(t[:], h2_sp[p * 128 : (p + 1) * 128, cs])
                        h2.append(t)
                    mg = []
                    for j in range(KP):
                        ps = psC2.tile([128, CH], F32, tag="m1", name="m1_ps",
                                       bufs=3)
                        for p in range(KP):
                            nc.tensor.matmul(
                                ps[:],
                                m1_w_r[p][:, j * 128 : (j + 1) * 128],
                                h2[p][:],
                                start=(p == 0), stop=(p == KP - 1),
                            )
                        g = pC2.tile([128, CH], F32R, tag=f"mg{j}", name=f"mg{j}",
                                     bufs=2)
                        nc.scalar.activation(g[:], ps[:], AF.Gelu, bias=m1b_t[j][:])
                        mg.append(g)
                    return mg

                def c2_m2(c, mg):
                    cs = slice(c * CH, (c + 1) * CH)
                    for o in range(KP):
                        x1 = pC2.tile([128, CH], F32, tag="x1l", name="x1l", bufs=3)
                        nc.sync.dma_start(x1[:], x1_sp[o * 128 : (o + 1) * 128, cs])
                        ps = psC2.tile([128, CH], F32, tag="m2", name="m2_ps",
                                       bufs=2)
                        for j in range(KP):
                            nc.tensor.matmul(
                                ps[:],
                                m2_w_r[j][:, o * 128 : (o + 1) * 128],
                                mg[j][:],
                                start=(j == 0), stop=(j == KP - 1),
                            )
                        mo = pC2.tile([128, CH], F32, tag="mo", name="mo", bufs=3)
                        nc.vector.scalar_tensor_tensor(
                            mo[:], ps[:], m2b_t[o][:], x1[:],
                            op0=ALU.add, op1=ALU.add,
                        )
                        nc.sync.dma_start(outT_d[o * 128 : (o + 1) * 128, cs], mo[:])

                for c in range(NCH):
                    c2_m2(c, c2_m1(c))

            _loop_ctx.close()

    nc.compile()
    return nc


_CACHE = {}


def _get_nc(apply_ln1_affine, apply_ln2_affine, loop_R=1):
    key = (apply_ln1_affine, apply_ln2_affine, loop_R)
    if key not in _CACHE:
        _CACHE[key] = _build(apply_ln1_affine, apply_ln2_affine, loop_R)
    return _CACHE[key]


def _round_f32r(x):
    """Round fp32 -> tf32-like (10 explicit mantissa bits, RNE)."""
    u = np.ascontiguousarray(x, np.float32).view(np.uint32)
    shift = 13
    bias = np.uint32((1 << (shift - 1)) - 1)
    lsb = (u >> np.uint32(shift)) & np.uint32(1)
    u2 = (u + bias + lsb) & np.uint32(~((1 << shift) - 1) & 0xFFFFFFFF)
    return u2.view(np.float32)


def kernel(**inputs):
    return _kernel_impl(inputs, loop_R=1)


def _kernel_impl(inputs, loop_R=1):
    x = np.ascontiguousarray(np.asarray(inputs["x"], np.float32))
    assert x.shape == (B, N, P), x.shape

    ln1_g = np.asarray(inputs["ln1_g"], np.float32)
    ln1_b = np.asarray(inputs["ln1_b"], np.float32)
    ln2_g = np.asarray(inputs["ln2_g"], np.float32)
    ln2_b = np.asarray(inputs["ln2_b"], np.float32)
    aff1 = not (np.all(ln1_g == 1.0) and np.all(ln1_b == 0.0))
    aff2 = not (np.all(ln2_g == 1.0) and np.all(ln2_b == 0.0))

    nc = _get_nc(aff1, aff2, loop_R)

    base = {
        "qk_wT": np.ascontiguousarray(np.asarray(inputs["qk_w"], np.float32).T),
        "v_wT": _round_f32r(np.asarray(inputs["v_w"], np.float32).T),
        "t1_wT": np.ascontiguousarray(np.asarray(inputs["t1_w"], np.float32).T),
        "t2_wT": np.ascontiguousarray(np.asarray(inputs["t2_w"], np.float32).T),
        "m1_wT": _round_f32r(np.asarray(inputs["m1_w"], np.float32).T),
        "m2_wT": _round_f32r(np.asarray(inputs["m2_w"], np.float32).T),
        "v_b": np.ascontiguousarray(np.asarray(inputs["v_b"], np.float32)),
        "t1_b": np.ascontiguousarray(np.asarray(inputs["t1_b"], np.float32)),
        "t2_b": np.ascontiguousarray(np.asarray(inputs["t2_b"], np.float32)),
        "m1_b": np.ascontiguousarray(np.asarray(inputs["m1_b"], np.float32)),
        "m2_b": np.ascontiguousarray(np.asarray(inputs["m2_b"], np.float32)),
    }
    if aff1:
        base["ln1_g"] = np.ascontiguousarray(ln1_g)
        base["ln1_b"] = np.ascontiguousarray(ln1_b)
    if aff2:
        base["ln2_g"] = np.ascontiguousarray(ln2_g)
        base["ln2_b"] = np.ascontiguousarray(ln2_b)

    in_maps = []
    for b in range(B):
        m = dict(base)
        m["xT"] = np.ascontiguousarray(x[b].T)
        in_maps.append(m)

    res = bass_utils.run_bass_kernel_spmd(nc, in_maps, core_ids=list(range(B)))
    out = np.empty((B, N, P), np.float32)
    for b in range(B):
        out[b] = res.results[b]["outT"].T
    return out


if __name__ == "__main__":
    import sys
    import time

    sys.path.insert(0, "/root/problem")
    import reference as refmod

    inputs = {k: np.asarray(v) for k, v in refmod.setup_inputs().items()}
    t0 = time.time()
    got = kernel(**inputs)
    print(f"kernel() took {time.time() - t0:.1f}s (incl compile)")
    t0 = time.time()
    got = kernel(**inputs)
    print(f"kernel() 2nd call {time.time() - t0:.1f}s")
    exp = np.asarray(refmod.reference(**inputs))
    err = np.abs(got - exp)
    scale = np.abs(exp).max()
    l2 = np.linalg.norm(got - exp) / np.linalg.norm(exp)
    print(f"absmax={err.max():.3e} scale-rel={err.max() / scale:.3e} L2rel={l2:.3e}")



# revision 3
# speedup vs baseline: 41.6988x; 41.6988x over previous
"""Trainium2 Bass kernel for nn_Enhanced_transformer (dense transformer block).

Strategy
--------
Data-parallel: batch B=8 -> one batch element per NeuronCore (8 cores), no
collectives. Per core, everything runs in channel-major ("transposed") layout
[channel-part, token-free], which makes every GEMM contraction land on the
partition dim with zero runtime transposes:

  h^T = LN1(x)^T           stats via ones-matmul partition reduction
  x_v^T = v_wT @ h^T       (f32r)        -> spill to DRAM
  x_q   = h^T' @ qk_wT     (fp32, [n-part, q-free])
  energy= x_q' @ x_q       (fp32, PSUM-accumulated across chunks)
  A1    = energy @ t1_wT   (fp32)  + t1_b (free-bcast)  -> gelu
  att2  = t2_wT' @ A1      (fp32)  + t2_b (part bias)   -> softmax -> f32r
  t_out^T = att' @ x_v^T   (f32r);  x1^T = t_out^T + x^T
  h2^T  = LN2(x1)^T        -> spill;  x1 -> spill
  m     = gelu(m1_wT' @ h2^T + m1_b);  out = m2_wT' @ m + m2_b + x1^T

dtypes: attention-logits chain exact fp32 (4 cyc/row but tiny FLOPs);
all big GEMMs float32r (tf32-like, 1 cyc/row at free>=256 == bf16 speed).
Expected absmax error vs fp32 reference ~7e-3 (simulated).

Host side: per-core transposes of x / weights; output transposed back.
"""

import numpy as np

import concourse.bass as bass
import concourse.tile as tile
from concourse import bacc, mybir
from concourse import bass_utils

F32 = mybir.dt.float32
F32R = mybir.dt.float32r
AF = mybir.ActivationFunctionType
ALU = mybir.AluOpType
AX = mybir.AxisListType

B, N, P = 8, 4096, 1024
P4 = P // 4          # 256
EPS = 1e-5
CH = 512             # token chunk
NCH = N // CH        # 8
KP = P // 128        # 8 channel tiles
KQ = P4 // 128       # 2


def _build(apply_ln1_affine: bool, apply_ln2_affine: bool, loop_R: int = 1):
    nc = bacc.Bacc("TRN2", target_bir_lowering=False, debug=False)

    # ---- DRAM I/O ----
    xT_d = nc.dram_tensor("xT", [P, N], F32, kind="ExternalInput").ap()
    qk_wT_d = nc.dram_tensor("qk_wT", [P, P4], F32, kind="ExternalInput").ap()
    v_wT_d = nc.dram_tensor("v_wT", [P, P], F32R, kind="ExternalInput").ap()
    t1_wT_d = nc.dram_tensor("t1_wT", [P4, P], F32, kind="ExternalInput").ap()
    t2_wT_d = nc.dram_tensor("t2_wT", [P4, P], F32, kind="ExternalInput").ap()
    m1_wT_d = nc.dram_tensor("m1_wT", [P, P], F32R, kind="ExternalInput").ap()
    m2_wT_d = nc.dram_tensor("m2_wT", [P, P], F32R, kind="ExternalInput").ap()
    v_b_d = nc.dram_tensor("v_b", [P], F32, kind="ExternalInput").ap()
    t1_b_d = nc.dram_tensor("t1_b", [P], F32, kind="ExternalInput").ap()
    t2_b_d = nc.dram_tensor("t2_b", [P], F32, kind="ExternalInput").ap()
    m1_b_d = nc.dram_tensor("m1_b", [P], F32, kind="ExternalInput").ap()
    m2_b_d = nc.dram_tensor("m2_b", [P], F32, kind="ExternalInput").ap()
    ln_d = {}
    if apply_ln1_affine:
        ln_d["ln1_g"] = nc.dram_tensor("ln1_g", [P], F32, kind="ExternalInput").ap()
        ln_d["ln1_b"] = nc.dram_tensor("ln1_b", [P], F32, kind="ExternalInput").ap()
    if apply_ln2_affine:
        ln_d["ln2_g"] = nc.dram_tensor("ln2_g", [P], F32, kind="ExternalInput").ap()
        ln_d["ln2_b"] = nc.dram_tensor("ln2_b", [P], F32, kind="ExternalInput").ap()
    outT_d = nc.dram_tensor("outT", [P, N], F32, kind="ExternalOutput").ap()

    def part_bias_tiles(pool, dram_ap, name):
        """[P] dram vector -> list of KP [128,1] per-partition SBUF tiles."""
        tiles = []
        for t in range(KP):
            bt = pool.tile([128, 1], F32, tag=f"{name}{t}", name=f"{name}{t}")
            nc.scalar.dma_start(bt[:], dram_ap[t * 128 : (t + 1) * 128])
            tiles.append(bt)
        return tiles

    with tile.TileContext(nc) as tc:
        with (
            tc.tile_pool(name="dram", bufs=1, space="DRAM") as dram_pool,
            tc.tile_pool(name="consts", bufs=1) as consts,
        ):
            xv_sp = dram_pool.tile([P, N], F32R, name="xv_sp")
            h2_sp = dram_pool.tile([P, N], F32R, name="h2_sp")
            x1_sp = dram_pool.tile([P, N], F32, name="x1_sp")
            att_sp = dram_pool.tile([P, P], F32R, name="att_sp")

            ones_f = consts.tile([128, 128], F32, tag="ones_f", name="ones_f")
            nc.vector.memset(ones_f[:], 1.0 / P)
            ones_r = consts.tile([128, 128], F32R, tag="ones_r", name="ones_r")
            nc.vector.tensor_copy(ones_r[:], ones_f[:])
            eps_t = consts.tile([128, 1], F32, tag="eps", name="eps_t")
            nc.vector.memset(eps_t[:], EPS)

            vb_t = part_bias_tiles(consts, v_b_d, "vb")
            t2b_t = part_bias_tiles(consts, t2_b_d, "t2b")
            m1b_t = part_bias_tiles(consts, m1_b_d, "m1b")
            m2b_t = part_bias_tiles(consts, m2_b_d, "m2b")
            # t1_b broadcast along partitions: [128, P] via stride-0 DMA
            t1b_bc = consts.tile([128, P], F32, tag="t1b_bc", name="t1b_bc")
            t1b_src = bass.AP(
                tensor=t1_b_d.tensor, offset=t1_b_d.offset,
                ap=[[0, 128], *t1_b_d.ap],
            )
            nc.scalar.dma_start(t1b_bc[:], t1b_src)
            ln_t = {}
            if apply_ln1_affine:
                ln_t["g1"] = part_bias_tiles(consts, ln_d["ln1_g"], "g1")
                ln_t["b1"] = part_bias_tiles(consts, ln_d["ln1_b"], "b1")
            if apply_ln2_affine:
                ln_t["g2"] = part_bias_tiles(consts, ln_d["ln2_g"], "g2")
                ln_t["b2"] = part_bias_tiles(consts, ln_d["ln2_b"], "b2")

            def ln_stats(psP, pP, x_tiles, sq_tiles, tag):
                """x_tiles: KP x [128,CH] fp32; sq_tiles f32r. -> (mu_b, rho_b)
                [128,CH] fp32, already broadcast to all partitions (full ones
                matrix in the matmul replicates the row-sums)."""
                ps_s = psP.tile([128, CH], F32, tag=f"{tag}_s", name=f"{tag}_s")
                ps_q = psP.tile([128, CH], F32, tag=f"{tag}_q", name=f"{tag}_q")
                for p in range(KP):
                    nc.tensor.matmul(
                        ps_s[:], ones_r[:], x_tiles[p][:],
                        start=(p == 0), stop=(p == KP - 1),
                    )
                for p in range(KP):
                    nc.tensor.matmul(
                        ps_q[:], ones_r[:], sq_tiles[p][:],
                        start=(p == 0), stop=(p == KP - 1),
                    )
                mu_b = pP.tile([128, CH], F32, tag=f"{tag}_mu", name=f"{tag}_mu")
                nc.vector.tensor_copy(mu_b[:], ps_s[:])
                var = pP.tile([128, CH], F32, tag=f"{tag}_var", name=f"{tag}_var")
                nc.vector.tensor_mul(var[:], mu_b[:], mu_b[:])
                nc.vector.tensor_tensor(var[:], ps_q[:], var[:], ALU.subtract)
                nc.scalar.activation(var[:], var[:], AF.Sqrt, bias=eps_t[:])
                rho_b = pP.tile([128, CH], F32, tag=f"{tag}_rho", name=f"{tag}_rho")
                nc.vector.reciprocal(rho_b[:], var[:])
                return mu_b, rho_b

            # Optional hardware repeat-loop for timing (test.py only).
            from contextlib import ExitStack as _ES
            _loop_ctx = _ES()
            if loop_R > 1:
                _loop_ctx.enter_context(tc.For_i(0, loop_R, 1))
            # ============ PHASE A: LN1, x_v, x_q, energy ============
            with tc.tile_pool(name="psE", bufs=1, space="PSUM") as psE:
                e_ps = [psE.tile([128, P4], F32, tag=f"e{i}", name=f"e_ps{i}")
                        for i in range(KQ)]
                with (
                    tc.tile_pool(name="wA", bufs=1) as wA,
                    tc.tile_pool(name="pA", bufs=1) as pA,
                    tc.tile_pool(name="psA", bufs=1, space="PSUM") as psA,
                ):
                    v_w_r = []
                    for p in range(KP):
                        wr = wA.tile([128, P], F32R, tag=f"vw{p}", name=f"vw{p}")
                        nc.scalar.dma_start(wr[:], v_wT_d[p * 128 : (p + 1) * 128, :])
                        v_w_r.append(wr)
                    qk_w_t = []
                    for p in range(KP):
                        wt = wA.tile([128, P4], F32, tag=f"qkw{p}", name=f"qkw{p}")
                        nc.scalar.dma_start(wt[:], qk_wT_d[p * 128 : (p + 1) * 128, :])
                        qk_w_t.append(wt)

                    for c in range(NCH):
                        cs = slice(c * CH, (c + 1) * CH)
                        xt = []
                        for p in range(KP):
                            t = pA.tile([128, CH], F32, tag=f"xt{p}", name=f"xt{p}",
                                        bufs=2)
                            nc.sync.dma_start(t[:], xT_d[p * 128 : (p + 1) * 128, cs])
                            xt.append(t)
                        xr, sq = [], []
                        for p in range(KP):
                            r = pA.tile([128, CH], F32R, tag=f"xr{p}", name=f"xr{p}")
                            nc.gpsimd.tensor_copy(r[:], xt[p][:])
                            xr.append(r)
                            s = pA.tile([128, CH], F32R, tag=f"sq{p}", name=f"sq{p}")
                            nc.scalar.activation(s[:], xt[p][:], AF.Square)
                            sq.append(s)
                        mu_b, rho_b = ln_stats(psA, pA, xr, sq, "st1")

                        h32, h_r = [], []
                        for p in range(KP):
                            h = pA.tile([128, CH], F32, tag=f"h32{p}", name=f"h32{p}",
                                        bufs=2)
                            nc.vector.tensor_tensor(h[:], xt[p][:], mu_b[:],
                                                    ALU.subtract)
                            nc.vector.tensor_mul(h[:], h[:], rho_b[:])
                            if apply_ln1_affine:
                                nc.scalar.activation(
                                    h[:], h[:], AF.Identity,
                                    bias=ln_t["b1"][p][:], scale=ln_t["g1"][p][:],
                                )
                            hr = pA.tile([128, CH], F32R, tag=f"hr{p}", name=f"hr{p}",
                                         bufs=2)
                            nc.scalar.activation(hr[:], h[:], AF.Copy)
                            h32.append(h)
                            h_r.append(hr)

                        # x_q chunk + energy accumulation
                        for ns in range(CH // 128):
                            ps = psA.tile([128, P4], F32, tag="xq", name="xq_ps",
                                          bufs=2)
                            for p in range(KP):
                                nc.tensor.matmul(
                                    ps[:],
                                    h32[p][:, ns * 128 : (ns + 1) * 128],
                                    qk_w_t[p][:],
                                    start=(p == 0), stop=(p == KP - 1),
                                )
                            xq = pA.tile([128, P4], F32, tag="xqs", name="xqs",
                                         bufs=3)
                            nc.vector.tensor_copy(xq[:], ps[:])
                            first = c == 0 and ns == 0
                            last = c == NCH - 1 and ns == CH // 128 - 1
                            for qh in range(KQ):
                                nc.tensor.matmul(
                                    e_ps[qh][:],
                                    xq[:, qh * 128 : (qh + 1) * 128],
                                    xq[:],
                                    start=first, stop=last,
                                    skip_group_check=True,
                                )

                        # x_v^T chunk
                        for o in range(KP):
                            ps = psA.tile([128, CH], F32, tag="xv", name="xv_ps",
                                          bufs=2)
                            for p in range(KP):
                                nc.tensor.matmul(
                                    ps[:], v_w_r[p][:, o * 128 : (o + 1) * 128],
                                    h_r[p][:], start=(p == 0), stop=(p == KP - 1),
                                )
                            xv = pA.tile([128, CH], F32R, tag="xvs", name="xvs",
                                         bufs=2)
                            nc.scalar.activation(
                                xv[:], ps[:], AF.Identity, bias=vb_t[o][:]
                            )
                            nc.sync.dma_start(
                                xv_sp[o * 128 : (o + 1) * 128, cs], xv[:]
                            )

                # ============ PHASE B: logits + softmax ============
                with (
                    tc.tile_pool(name="wB", bufs=1) as wB,
                    tc.tile_pool(name="pB", bufs=1) as pB,
                    tc.tile_pool(name="psB", bufs=1, space="PSUM") as psB,
                ):
                    t1_w_t, t2_w_t = [], []
                    for qh in range(KQ):
                        wt = wB.tile([128, P], F32, tag=f"t1w{qh}", name=f"t1w{qh}")
                        nc.scalar.dma_start(wt[:],
                                            t1_wT_d[qh * 128 : (qh + 1) * 128, :])
                        t1_w_t.append(wt)
                        wt2 = wB.tile([128, P], F32, tag=f"t2w{qh}", name=f"t2w{qh}")
                        nc.scalar.dma_start(wt2[:],
                                            t2_wT_d[qh * 128 : (qh + 1) * 128, :])
                        t2_w_t.append(wt2)
                    energy_sb = []
                    for qh in range(KQ):
                        e = wB.tile([128, P4], F32, tag=f"esb{qh}", name=f"esb{qh}")
                        nc.vector.tensor_copy(e[:], e_ps[qh][:])
                        energy_sb.append(e)

                    # A1[b, a] = sum_q energy[q,b] t1_wT[q,a]; +t1_b[a]; gelu
                    a1g = []
                    for bh in range(KQ):
                        a1 = pB.tile([128, P], F32, tag=f"a1_{bh}", name=f"a1_{bh}")
                        for oc in range(P // 512):
                            ps = psB.tile([128, 512], F32, tag="a1", name="a1_ps",
                                          bufs=2)
                            for qh in range(KQ):
                                nc.tensor.matmul(
                                    ps[:],
                                    energy_sb[qh][:, bh * 128 : (bh + 1) * 128],
                                    t1_w_t[qh][:, oc * 512 : (oc + 1) * 512],
                                    start=(qh == 0), stop=(qh == KQ - 1),
                                )
                            nc.vector.tensor_tensor(
                                a1[:, oc * 512 : (oc + 1) * 512], ps[:],
                                t1b_bc[:, oc * 512 : (oc + 1) * 512], ALU.add,
                            )
                        ag = wB.tile([128, P], F32, tag=f"a1g{bh}", name=f"a1g{bh}")
                        nc.scalar.activation(ag[:], a1[:], AF.Gelu)
                        a1g.append(ag)

                    # att2 + softmax -> att_r (f32r)
                    for o in range(KP):
                        att2 = pB.tile([128, P], F32, tag="att2", name="att2",
                                       bufs=2)
                        for kc in range(P // 512):
                            ps = psB.tile([128, 512], F32, tag="a2", name="a2_ps",
                                          bufs=2)
                            for ph in range(KQ):
                                nc.tensor.matmul(
                                    ps[:],
                                    t2_w_t[ph][:, o * 128 : (o + 1) * 128],
                                    a1g[ph][:, kc * 512 : (kc + 1) * 512],
                                    start=(ph == 0), stop=(ph == KQ - 1),
                                )
                            nc.scalar.activation(
                                att2[:, kc * 512 : (kc + 1) * 512], ps[:],
                                AF.Identity, bias=t2b_t[o][:],
                            )
                        negmax = pB.tile([128, 1], F32, tag="negmax", name="negmax",
                                         bufs=2)
                        nc.vector.tensor_reduce(
                            negmax[:], att2[:], axis=AX.X, op=ALU.max, negate=True
                        )
                        esum = pB.tile([128, 1], F32, tag="esum", name="esum",
                                       bufs=2)
                        expv = pB.tile([128, P], F32, tag="expv", name="expv",
                                       bufs=2)
                        nc.scalar.activation(
                            expv[:], att2[:], AF.Exp, bias=negmax[:],
                            accum_out=esum[:],
                        )
                        rec = pB.tile([128, 1], F32, tag="rec", name="rec", bufs=2)
                        nc.vector.reciprocal(rec[:], esum[:])
                        ar = pB.tile([128, P], F32R, tag="att_t", name="att_t",
                                     bufs=2)
                        nc.vector.tensor_scalar_mul(ar[:], expv[:], rec[:])
                        nc.sync.dma_start(
                            att_sp[o * 128 : (o + 1) * 128, :], ar[:]
                        )

            # ============ PHASE C1: t_out, x1, LN2, h2 ============
            with (
                tc.tile_pool(name="pC1", bufs=1) as pC,
                tc.tile_pool(name="psC1", bufs=1, space="PSUM") as psC,
            ):
                att_r = []
                for p in range(KP):
                    ar = pC.tile([128, P], F32R, tag=f"att{p}", name=f"att{p}")
                    nc.scalar.dma_start(ar[:], att_sp[p * 128 : (p + 1) * 128, :])
                    att_r.append(ar)

                def c1_tout_q(c, q, xt, xv):
                    cs = slice(c * CH, (c + 1) * CH)
                    nb = 1
                    ps = psC.tile([128, CH], F32, tag="tout", name="tout_ps",
                                  bufs=4)
                    for p in range(KP):
                        nc.tensor.matmul(
                            ps[:],
                            att_r[p][:, q * 128 : (q + 1) * 128],
                            xv[p][:],
                            start=(p == 0), stop=(p == KP - 1),
                        )
                    x1 = pC.tile([128, CH], F32, tag=f"x1{q}", name=f"x1{q}",
                                 bufs=nb)
                    nc.vector.tensor_tensor(x1[:], ps[:], xt[q][:], ALU.add)
                    nc.sync.dma_start(x1_sp[q * 128 : (q + 1) * 128, cs], x1[:])
                    r = pC.tile([128, CH], F32R, tag=f"x1r{q}", name=f"x1r{q}",
                                bufs=nb)
                    nc.gpsimd.tensor_copy(r[:], x1[:])
                    sq = pC.tile([128, CH], F32R, tag=f"sq2{q}", name=f"sq2{q}",
                                 bufs=nb)
                    nc.scalar.activation(sq[:], x1[:], AF.Square)
                    return x1, r, sq

                def c1_stats_h2(c, x1f, x1r, sq2):
                    cs = slice(c * CH, (c + 1) * CH)
                    mu2, rho2 = ln_stats(psC, pC, x1r, sq2, "st2")
                    for p in range(KP):
                        nc.vector.tensor_tensor(x1f[p][:], x1f[p][:], mu2[:],
                                                ALU.subtract)
                        h2r = pC.tile([128, CH], F32R, tag=f"h2r{p}",
                                      name=f"h2r{p}", bufs=2)
                        nc.vector.tensor_mul(h2r[:], x1f[p][:], rho2[:])
                        if apply_ln2_affine:
                            nc.scalar.activation(
                                h2r[:], h2r[:], AF.Identity,
                                bias=ln_t["b2"][p][:], scale=ln_t["g2"][p][:],
                            )
                        nc.sync.dma_start(
                            h2_sp[p * 128 : (p + 1) * 128, cs], h2r[:]
                        )

                for c in range(NCH):
                    cs = slice(c * CH, (c + 1) * CH)
                    xt = []
                    for p in range(KP):
                        t = pC.tile([128, CH], F32, tag=f"xt{p}", name=f"xt{p}",
                                    bufs=2)
                        nc.sync.dma_start(t[:], xT_d[p * 128 : (p + 1) * 128, cs])
                        xt.append(t)
                    xv = []
                    for p in range(KP):
                        t = pC.tile([128, CH], F32R, tag=f"xv{p}", name=f"xv{p}",
                                    bufs=2)
                        nc.sync.dma_start(t[:],
                                          xv_sp[p * 128 : (p + 1) * 128, cs])
                        xv.append(t)

                    x1f, x1r, sq2 = [], [], []
                    for q in range(KP):
                        a, b_, d = c1_tout_q(c, q, xt, xv)
                        x1f.append(a); x1r.append(b_); sq2.append(d)
                    c1_stats_h2(c, x1f, x1r, sq2)

            # ============ PHASE C2: MLP + final residual ============
            with (
                tc.tile_pool(name="wC2", bufs=1) as wC,
                tc.tile_pool(name="pC2", bufs=1) as pC2,
                tc.tile_pool(name="psC2", bufs=1, space="PSUM") as psC2,
            ):
                m1_w_r, m2_w_r = [], []
                for p in range(KP):
                    wr = wC.tile([128, P], F32R, tag=f"m1w{p}", name=f"m1w{p}")
                    nc.scalar.dma_start(wr[:], m1_wT_d[p * 128 : (p + 1) * 128, :])
                    m1_w_r.append(wr)
                for p in range(KP):
                    wr = wC.tile([128, P], F32R, tag=f"m2w{p}", name=f"m2w{p}")
                    nc.sync.dma_start(wr[:], m2_wT_d[p * 128 : (p + 1) * 128, :])
                    m2_w_r.append(wr)

                def c2_m1(c):
                    cs = slice(c * CH, (c + 1) * CH)
                    h2 = []
                    for p in range(KP):
                        t = pC2.tile([128, CH], F32R, tag=f"h2{p}", name=f"h2{p}",
                                     bufs=2)
                        nc.sync.dma_start(t[:], h2_sp[p * 128 : (p + 1) * 128, cs])
                        h2.append(t)
                    mg = []
                    for j in range(KP):
                        ps = psC2.tile([128, CH], F32, tag="m1", name="m1_ps",
                                       bufs=3)
                        for p in range(KP):
                            nc.tensor.matmul(
                                ps[:],
                                m1_w_r[p][:, j * 128 : (j + 1) * 128],
                                h2[p][:],
                                start=(p == 0), stop=(p == KP - 1),
                            )
                        g = pC2.tile([128, CH], F32R, tag=f"mg{j}", name=f"mg{j}",
                                     bufs=2)
                        nc.scalar.activation(g[:], ps[:], AF.Gelu, bias=m1b_t[j][:])
                        mg.append(g)
                    return mg

                def c2_m2(c, mg):
                    cs = slice(c * CH, (c + 1) * CH)
                    for o in range(KP):
                        x1 = pC2.tile([128, CH], F32, tag="x1l", name="x1l", bufs=3)
                        nc.sync.dma_start(x1[:], x1_sp[o * 128 : (o + 1) * 128, cs])
                        ps = psC2.tile([128, CH], F32, tag="m2", name="m2_ps",
                                       bufs=2)
                        for j in range(KP):
                            nc.tensor.matmul(
                                ps[:],
                                m2_w_r[j][:, o * 128 : (o + 1) * 128],
                                mg[j][:],
                                start=(j == 0), stop=(j == KP - 1),
                            )
                        mo = pC2.tile([128, CH], F32, tag="mo", name="mo", bufs=3)
                        nc.vector.scalar_tensor_tensor(
                            mo[:], ps[:], m2b_t[o][:], x1[:],
                            op0=ALU.add, op1=ALU.add,
                        )
                        nc.sync.dma_start(outT_d[o * 128 : (o + 1) * 128, cs], mo[:])

                for c in range(NCH):
                    c2_m2(c, c2_m1(c))

            _loop_ctx.close()

    nc.compile()
    return nc


_CACHE = {}


def _get_nc(apply_ln1_affine, apply_ln2_affine, loop_R=1):
    key = (apply_ln1_affine, apply_ln2_affine, loop_R)
    if key not in _CACHE:
        _CACHE[key] = _build(apply_ln1_affine, apply_ln2_affine, loop_R)
    return _CACHE[key]


def _round_f32r(x):
    """Round fp32 -> tf32-like (10 explicit mantissa bits, RNE)."""
    u = np.ascontiguousarray(x, np.float32).view(np.uint32)
    shift = 13
    bias = np.uint32((1 << (shift - 1)) - 1)
    lsb = (u >> np.uint32(shift)) & np.uint32(1)
    u2 = (u + bias + lsb) & np.uint32(~((1 << shift) - 1) & 0xFFFFFFFF)
    return u2.view(np.float32)


def kernel(**inputs):
    return _kernel_impl(inputs, loop_R=1)


def _kernel_impl(inputs, loop_R=1, trace=False, tmpdir=None):
    x = np.ascontiguousarray(np.asarray(inputs["x"], np.float32))
    assert x.shape == (B, N, P), x.shape

    ln1_g = np.asarray(inputs["ln1_g"], np.float32)
    ln1_b = np.asarray(inputs["ln1_b"], np.float32)
    ln2_g = np.asarray(inputs["ln2_g"], np.float32)
    ln2_b = np.asarray(inputs["ln2_b"], np.float32)
    aff1 = not (np.all(ln1_g == 1.0) and np.all(ln1_b == 0.0))
    aff2 = not (np.all(ln2_g == 1.0) and np.all(ln2_b == 0.0))

    nc = _get_nc(aff1, aff2, loop_R)

    base = {
        "qk_wT": np.ascontiguousarray(np.asarray(inputs["qk_w"], np.float32).T),
        "v_wT": _round_f32r(np.asarray(inputs["v_w"], np.float32).T),
        "t1_wT": np.ascontiguousarray(np.asarray(inputs["t1_w"], np.float32).T),
        "t2_wT": np.ascontiguousarray(np.asarray(inputs["t2_w"], np.float32).T),
        "m1_wT": _round_f32r(np.asarray(inputs["m1_w"], np.float32).T),
        "m2_wT": _round_f32r(np.asarray(inputs["m2_w"], np.float32).T),
        "v_b": np.ascontiguousarray(np.asarray(inputs["v_b"], np.float32)),
        "t1_b": np.ascontiguousarray(np.asarray(inputs["t1_b"], np.float32)),
        "t2_b": np.ascontiguousarray(np.asarray(inputs["t2_b"], np.float32)),
        "m1_b": np.ascontiguousarray(np.asarray(inputs["m1_b"], np.float32)),
        "m2_b": np.ascontiguousarray(np.asarray(inputs["m2_b"], np.float32)),
    }
    if aff1:
        base["ln1_g"] = np.ascontiguousarray(ln1_g)
        base["ln1_b"] = np.ascontiguousarray(ln1_b)
    if aff2:
        base["ln2_g"] = np.ascontiguousarray(ln2_g)
        base["ln2_b"] = np.ascontiguousarray(ln2_b)

    in_maps = []
    for b in range(B):
        m = dict(base)
        m["xT"] = np.ascontiguousarray(x[b].T)
        in_maps.append(m)

    res = bass_utils.run_bass_kernel_spmd(
        nc, in_maps, core_ids=list(range(B)), trace=trace, tmpdir=tmpdir
    )
    out = np.empty((B, N, P), np.float32)
    for b in range(B):
        out[b] = res.results[b]["outT"].T
    if trace:
        return out, res
    return out


if __name__ == "__main__":
    import sys
    import time

    sys.path.insert(0, "/root/problem")
    import reference as refmod

    inputs = {k: np.asarray(v) for k, v in refmod.setup_inputs().items()}
    t0 = time.time()
    got = kernel(**inputs)
    print(f"kernel() took {time.time() - t0:.1f}s (incl compile)")
    t0 = time.time()
    got = kernel(**inputs)
    print(f"kernel() 2nd call {time.time() - t0:.1f}s")
    exp = np.asarray(refmod.reference(**inputs))
    err = np.abs(got - exp)
    scale = np.abs(exp).max()
    l2 = np.linalg.norm(got - exp) / np.linalg.norm(exp)
    print(f"absmax={err.max():.3e} scale-rel={err.max() / scale:.3e} L2rel={l2:.3e}")

